# revision 30
# baseline (speedup 1.0000x reference)
"""BN(train) -> binarize -> conv1d(K=7,pad=3) -> alpha-scale -> maxpool2 on 8 trn2 cores.

Data-parallel over batch N: each core owns 8 samples. BN statistics are
computed per-core with bn_stats/bn_aggr; per-core (mean, E[x^2]) partials
([128,2] fp32) are exchanged with a tiny AllGather, then folded locally:
an exact fp32 matmul against a stacked-identity matrix sums the two
partition halves and broadcasts the result to all 128 partitions in one
shot. alpha is folded into the conv weights on the host.

Per-core layout: each sample is held in SBUF as [128, 4102] fp32 where
partitions 0-63 = channels for L 0..4095 (+halo) and partitions 64-127 =
channels for L 4096..8191 (+halo). The binarize is one ScalarE Sign op
whose output access pattern de-interleaves even/odd positions
(xb = [evens | odds]), so the conv can accumulate even-position and
odd-position outputs into separate PSUM banks from contiguous rhs slices.
The maxpool then collapses to a single VectorE tensor_tensor max over a
bank pair, written directly as bf16; outputs are stored as bf16 and
widened to fp32 on the host.
"""

import numpy as np
from contextlib import ExitStack

N, CIN, L = 64, 64, 8192
COUT, K = 128, 7
NCORES = 8
NS = N // NCORES          # samples per core
HALF = L // 2             # 4096
PAD = K // 2              # 3
WCOLS = HALF + 2 * PAD    # 4102
PHALF = WCOLS // 2        # 2051 (parity-split width incl. halo)
T = 512                   # conv output cols per matmul (= PSUM bank)
NT = HALF // (2 * T)      # 4 tile groups per half (each covers 1024 cols)
POOL_HALF = HALF // 2     # 2048 pooled cols per half
EPS = 1e-5

_CACHE = {}


def _build():
    import concourse.bass as bass
    import concourse.tile as tile
    from concourse import mybir

    f32 = mybir.dt.float32
    bf16 = mybir.dt.bfloat16
    Alu = mybir.AluOpType
    Act = mybir.ActivationFunctionType

    nc = bass.Bass()
    # I pre-transposed on host to [NS, 128, HALF]: row h*64+c holds
    # I[n, c, h*HALF:(h+1)*HALF]. 2D regular 128-row DRAM loads hit
    # fabric rate on the HWDGE path.
    I_h = nc.declare_dram_parameter("I", [NS, 128, HALF], f32, isOutput=False)
    # host-prepped: [128, K, COUT] bf16, channels duplicated on both halves,
    # alpha folded in
    Wb_h = nc.declare_dram_parameter("Wb", [128, K, COUT], bf16, isOutput=False)
    # gamma/beta duplicated to [128] on host
    g_h = nc.declare_dram_parameter("gamma2", [128], f32, isOutput=False)
    be_h = nc.declare_dram_parameter("beta2", [128], f32, isOutput=False)
    # [128,128] fp32: eye4[c,m] = 1 iff c % 64 == m % 64 (sums the two
    # partition halves and broadcasts to both halves, exactly, in fp32)
    eye_h = nc.declare_dram_parameter("eye4", [128, 128], f32, isOutput=False)
    out_h = nc.declare_dram_parameter("out", [NS, COUT, HALF], bf16, isOutput=True)

    with ExitStack() as ctx:
        tc = ctx.enter_context(tile.TileContext(nc))
        singles = ctx.enter_context(tc.tile_pool(name="singles", bufs=1))
        ibufs = ctx.enter_context(tc.tile_pool(name="ibufs", bufs=NS))
        xbs = ctx.enter_context(tc.tile_pool(name="xbs", bufs=3))
        psums = ctx.enter_context(tc.tile_pool(name="psums", bufs=2, space="PSUM"))
        stages = ctx.enter_context(tc.tile_pool(name="stages", bufs=4))
        dram = ctx.enter_context(tc.tile_pool(name="dram", bufs=1, space="DRAM"))

        # --- warmup AllGather, first thing on the CC stream: pays the
        # ncfw cold-boot + cross-core execution-start skew in the shadow of
        # the input loads. The input DRAM tile is deliberately
        # uninitialized — the gathered values are never read — so the
        # doorbell has no dependencies and rings immediately.
        agw_in = dram.tile([128, 2], f32)
        agw_out = dram.tile([NCORES * 128, 2], f32, addr_space="Shared")
        nc.gpsimd.collective_compute(
            "AllGather",
            Alu.bypass,
            replica_groups=[list(range(NCORES))],
            ins=[agw_in[:]],
            outs=[agw_out[:]],
        )

        eps_t = singles.tile([128, 1], f32)
        nc.vector.memset(eps_t[:], EPS)

        # --- phase 1: load + per-partition stats -------------------------
        # 512 KB chunks so the first stats tiles land early and the tail
        # past the last chunk is short. The 64 stat tiles are split
        # between VectorE (bn_stats, 44 tiles) and the otherwise-idle
        # ScalarE (20 tiles via two activation passes whose accum_out
        # gives per-partition sums: Square -> sum(x^2), Copy -> sum(x)).
        # ScalarE issues no load DMAs (loads ride the sync HWDGE and
        # gpsimd SWDGE rings), so its stats ACTs start immediately; its
        # share is front-loaded on early samples since it is the slower
        # engine per tile and late-sample data lands last.
        NSC_PER = [4, 4, 3, 3, 2, 2, 1, 1]     # scalar-owned tiles/sample
        NSC = sum(NSC_PER)                      # 20
        NDVE = NS * 8 - NSC                     # 44
        stats = singles.tile([128, NDVE, 6], f32)
        sq_acc = singles.tile([128, 2, NSC], f32)   # [;(sumsq,sum); tile]
        scst = ctx.enter_context(tc.tile_pool(name="scst", bufs=2))
        # all load issues first, so neither HWDGE ring is ever starved
        # behind compute instructions on its issuing engine
        ibs = []
        for n in range(NS):
            ib = ibufs.tile([128, WCOLS], f32, name=f"ib{n}", tag="ib")
            ibs.append(ib)
            # small chunks for the first two samples (early stats start),
            # 1 MB chunks for the bulk (better DMA efficiency)
            bounds = [0, 1024, 2048, 3072, 4096] if n < 2 else [0, 2048, 4096]
            for h, (b0, b1) in enumerate(zip(bounds, bounds[1:])):
                eng = nc.sync if (n + h) % 2 == 0 else nc.gpsimd
                eng.dma_start(
                    out=ib[:, PAD + b0 : PAD + b1],
                    in_=I_h[n, :, b0:b1],
                )
        # constants after the load issues (they are not needed until the
        # stats fold / conv phase)
        wsb = singles.tile([128, K, COUT], bf16)
        nc.gpsimd.dma_start(out=wsb[:], in_=Wb_h[:])
        eye4 = singles.tile([128, 128], f32)
        nc.gpsimd.dma_start(out=eye4[:], in_=eye_h[:])
        gam = singles.tile([128, 1], f32)
        bet = singles.tile([128, 1], f32)
        nc.gpsimd.dma_start(out=gam[:], in_=g_h[:].rearrange("(c o) -> c o", o=1))
        nc.gpsimd.dma_start(out=bet[:], in_=be_h[:].rearrange("(c o) -> c o", o=1))
        for n in range(NS):
            ib = ibs[n]
            # halo: lo rows need positions 4096..4098 (hi rows, first 3),
            # hi rows need 4093..4095 (lo rows, last 3)
            nc.gpsimd.dma_start(
                out=ib[0:CIN, WCOLS - PAD : WCOLS], in_=I_h[n, CIN:128, 0:PAD]
            )
            nc.gpsimd.dma_start(
                out=ib[CIN:128, 0:PAD], in_=I_h[n, 0:CIN, HALF - PAD : HALF]
            )
        idve = isc = 0
        for n in range(NS):
            ib = ibs[n]
            nsc = NSC_PER[n]
            for c in range(8):
                tl = ib[:, PAD + c * T : PAD + (c + 1) * T]
                if c < nsc:
                    scr = scst.tile([128, T], bf16, name="scr")
                    nc.scalar.activation(
                        out=scr[:], in_=tl, func=Act.Copy,
                        accum_out=sq_acc[:, 0, isc : isc + 1],
                    )
                    nc.scalar.activation(
                        out=scr[:], in_=tl, func=Act.Square,
                        accum_out=sq_acc[:, 1, isc : isc + 1],
                    )
                    isc += 1
                else:
                    nc.vector.bn_stats(out=stats[:, idve, :], in_=tl)
                    idve += 1

        # --- fold per-core stats -> raw (sum, sumsq) [128,2], AllGather --
        mv = singles.tile([128, 2], f32)
        nc.vector.bn_aggr(out=mv[:], in_=stats[:])
        tmp0 = singles.tile([128, 1], f32)
        nc.vector.tensor_tensor(
            out=tmp0[:], in0=mv[:, 0:1], in1=mv[:, 0:1], op=Alu.mult
        )
        # mv becomes (mean, E[x^2]) over the DVE-owned tiles
        nc.vector.tensor_tensor(
            out=mv[:, 1:2], in0=mv[:, 1:2], in1=tmp0[:], op=Alu.add
        )
        qs_sc = singles.tile([128, 2], f32)
        nc.vector.tensor_reduce(
            out=qs_sc[:], in_=sq_acc[:], axis=mybir.AxisListType.X, op=Alu.add
        )
        # ms = raw (sum, sumsq) = (mean, Ex2)*Ndve*T + (sum_sc, sumsq_sc)
        ms = singles.tile([128, 2], f32)
        nc.vector.scalar_tensor_tensor(
            out=ms[:],
            in0=mv[:],
            scalar=float(NDVE * T),
            in1=qs_sc[:],
            op0=Alu.mult,
            op1=Alu.add,
        )
        ag_in = dram.tile([128, 2], f32)
        ag_out = dram.tile([NCORES * 128, 2], f32, addr_space="Shared")
        nc.sync.dma_start(out=ag_in[:], in_=ms[:])
        nc.gpsimd.collective_compute(
            "AllGather",
            Alu.bypass,
            replica_groups=[list(range(NCORES))],
            ins=[ag_in[:]],
            outs=[ag_out[:]],
        )
        ag_sb = singles.tile([128, 2, NCORES], f32)
        nc.sync.dma_start(
            out=ag_sb[:], in_=ag_out[:].rearrange("(r p) t -> p t r", r=NCORES)
        )
        s8 = singles.tile([128, 2], f32)
        nc.vector.tensor_reduce(
            out=s8[:], in_=ag_sb[:], axis=mybir.AxisListType.X, op=Alu.add
        )
        # exact fp32 fold of the two partition halves, broadcast to both.
        # Borrows a conv-pool PSUM slot; it is read (and released) before
        # the first conv matmul needs it.
        q_tile = psums.tile([COUT, T], f32, name="ps_loe")
        q_ps = q_tile[:, 0:2]
        nc.tensor.matmul(q_ps, eye4[:], s8[:], start=True, stop=True)

        # --- global stats -> binarize scale/bias -------------------------
        q = singles.tile([128, 2], f32)
        # q = (mean, E[x^2]) from the raw global sums over N*L elements
        nc.vector.tensor_scalar_mul(q[:], q_ps, 1.0 / float(N * L))
        sb = singles.tile([128, 2], f32)
        tmp = singles.tile([128, 2], f32)
        var = singles.tile([128, 1], f32)
        rstd = singles.tile([128, 1], f32)
        nc.vector.tensor_tensor(
            out=tmp[:, 0:1], in0=q[:, 0:1], in1=q[:, 0:1], op=Alu.mult
        )
        nc.vector.tensor_tensor(
            out=var[:], in0=q[:, 1:2], in1=tmp[:, 0:1], op=Alu.subtract
        )
        nc.scalar.activation(
            out=rstd[:], in_=var[:], func=Act.Sqrt, bias=eps_t[:], scale=1.0
        )
        nc.vector.reciprocal(out=rstd[:], in_=rstd[:])
        # s = gamma*rstd ; b = beta - mean*s
        nc.vector.tensor_tensor(out=sb[:, 0:1], in0=gam[:], in1=rstd[:], op=Alu.mult)
        nc.vector.tensor_tensor(
            out=tmp[:, 1:2], in0=q[:, 0:1], in1=sb[:, 0:1], op=Alu.mult
        )
        nc.vector.tensor_tensor(
            out=sb[:, 1:2], in0=bet[:], in1=tmp[:, 1:2], op=Alu.subtract
        )

        # --- phase 2: binarize (parity-split) + conv + pool --------------
        for n in range(NS):
            ib = ibs[n]
            # xb layout per partition: [evens (2051) | odds (2051)], bf16.
            # Activations use stride-2 single-dim input views and
            # contiguous outputs (a 2-element inner AP dim costs ~9
            # cycles/row on the engines; a flat strided dim is full rate).
            # Sample 0 is on the critical path out of the stats fold, so
            # its binarize is split into E/O column chunks — the first
            # conv tiles only need the leading columns of each parity.
            xb = xbs.tile([128, 2 * PHALF], bf16, name="xb")
            ib_par = ib[:].rearrange("p (w two) -> p two w", two=2)
            csplits = [0, 516, 1028, 1540, PHALF] if n == 0 else [0, PHALF]
            for c0, c1 in zip(csplits, csplits[1:]):
                for par in range(2):
                    nc.scalar.activation(
                        out=xb[:, par * PHALF + c0 : par * PHALF + c1],
                        in_=ib_par[:, par, c0:c1],
                        func=Act.Sign,
                        bias=sb[:, 1:2],
                        scale=sb[:, 0:1],
                    )
            # zero the off-the-end pads (binarize mapped them to +-1):
            # lo rows: padded p in {0,1,2} -> E0, E1 (cols 0:2) and O0 (2051)
            # hi rows: padded p in {4099,4100,4101} -> E2050 (2050) and
            #          O2049, O2050 (cols 4100:4102)
            nc.gpsimd.memset(xb[0:CIN, 0:2], 0.0)
            nc.gpsimd.memset(xb[0:CIN, PHALF : PHALF + 1], 0.0)
            nc.gpsimd.memset(xb[CIN:128, PHALF - 1 : PHALF], 0.0)
            nc.gpsimd.memset(xb[CIN:128, 2 * PHALF - 2 : 2 * PHALF], 0.0)

            for j in range(NT):
                m0 = j * T
                ps_loe = psums.tile([COUT, T], f32, name="ps_loe")
                ps_loo = psums.tile([COUT, T], f32, name="ps_loo")
                ps_hie = psums.tile([COUT, T], f32, name="ps_hie")
                ps_hio = psums.tile([COUT, T], f32, name="ps_hio")
                for k in range(K):
                    st = k == 0
                    sp = k == K - 1
                    # even outputs c=2m: tap k reads parity (k%2) at
                    # offset k//2; odd outputs c=2m+1: parity (k+1)%2 at
                    # offset (k+1)//2. Same-weight matmuls are adjacent
                    # (e then o per row group) so the weight load can be
                    # shared/dedued by the backend.
                    eo = (k % 2) * PHALF + k // 2 + m0
                    oo = ((k + 1) % 2) * PHALF + (k + 1) // 2 + m0
                    nc.tensor.matmul(
                        ps_loe[:], wsb[0:CIN, k, :], xb[0:CIN, eo : eo + T],
                        start=st, stop=sp,
                    )
                    nc.tensor.matmul(
                        ps_loo[:], wsb[0:CIN, k, :], xb[0:CIN, oo : oo + T],
                        start=st, stop=sp,
                    )
                    nc.tensor.matmul(
                        ps_hie[:], wsb[CIN:128, k, :], xb[CIN:128, eo : eo + T],
                        start=st, stop=sp,
                    )
                    nc.tensor.matmul(
                        ps_hio[:], wsb[CIN:128, k, :], xb[CIN:128, oo : oo + T],
                        start=st, stop=sp,
                    )
                # walrus only allows one PSUM input per DVE op: ScalarE
                # evacuates the even banks (ACT is fast at PSUM reads), the
                # max then reads one SBUF + one PSUM operand on VectorE.
                ev_lo = stages.tile([COUT, T], f32, name="ev_lo")
                ev_hi = stages.tile([COUT, T], f32, name="ev_hi")
                if n == 0:
                    # ScalarE is still busy binarizing early on; keep the
                    # first sample's bank recycling off its queue
                    nc.vector.tensor_copy(out=ev_lo[:], in_=ps_loe[:])
                    nc.vector.tensor_copy(out=ev_hi[:], in_=ps_hie[:])
                else:
                    nc.scalar.activation(
                        out=ev_lo[:], in_=ps_loe[:], func=Act.Copy
                    )
                    nc.scalar.activation(
                        out=ev_hi[:], in_=ps_hie[:], func=Act.Copy
                    )
                st_lo = stages.tile([COUT, T], bf16, name="st_lo")
                st_hi = stages.tile([COUT, T], bf16, name="st_hi")
                nc.vector.tensor_tensor(
                    out=st_lo[:], in0=ev_lo[:], in1=ps_loo[:], op=Alu.max
                )
                nc.vector.tensor_tensor(
                    out=st_hi[:], in0=ev_hi[:], in1=ps_hio[:], op=Alu.max
                )
                nc.sync.dma_start(
                    out=out_h[n, :, j * T : (j + 1) * T], in_=st_lo[:]
                )
                nc.sync.dma_start(
                    out=out_h[n, :, POOL_HALF + j * T : POOL_HALF + (j + 1) * T],
                    in_=st_hi[:],
                )

    return nc


def _split_multi_waits(nc):
    """walrus codegen only supports one sync-wait command per instruction;
    the TileContext exit drain carries several. Split the extras onto NOPs
    inserted immediately before the offending instruction."""
    import bass_rust
    from concourse import mybir

    for f in nc.m.functions:
        for bb in f.blocks:
            idx = 0
            while idx < len(bb.instructions):
                ins = bb.instructions[idx]
                si = ins.sync_info
                if si is not None and si.on_wait and len(si.on_wait) > 1:
                    waits = list(si.on_wait)
                    keep, rest = waits[-1], waits[:-1]
                    ins.sync_info = bass_rust.SyncInfo(
                        on_wait=[keep], on_update=list(si.on_update or [])
                    )
                    new_insts = []
                    for w in rest:
                        nop = mybir.InstNoOp(
                            name=nc.get_next_instruction_name(), ins=[], outs=[]
                        )
                        nop.engine = ins.engine
                        nop.sync_info = bass_rust.SyncInfo(on_wait=[w], on_update=[])
                        new_insts.append(nop)
                    for j, nop in enumerate(new_insts):
                        bb.instructions.insert(idx + j, nop)
                    idx += len(new_insts)
                idx += 1


def _get_nc(split=True):
    key = ("nc", split)
    if key not in _CACHE:
        nc = _build()
        if split:
            _split_multi_waits(nc)
        _CACHE[key] = nc
    return _CACHE[key]


def _make_in_maps(I, gamma, beta, W, alpha):
    import ml_dtypes

    I = np.asarray(I, dtype=np.float32)
    gamma = np.ascontiguousarray(np.asarray(gamma, dtype=np.float32))
    beta = np.ascontiguousarray(np.asarray(beta, dtype=np.float32))
    W = np.asarray(W, dtype=np.float32)
    alpha = np.asarray(alpha, dtype=np.float32)

    # stack the two L-halves on the partition axis: [N, 128, HALF]
    I2 = np.ascontiguousarray(
        I.reshape(N, CIN, 2, HALF).transpose(0, 2, 1, 3).reshape(N, 128, HALF)
    )
    # fold alpha into the weights, arrange [CIN, K, COUT], duplicate the
    # channel block on both partition halves, cast bf16
    Wt = (W * alpha.reshape(COUT, 1, 1)).transpose(1, 2, 0)  # [CIN, K, COUT]
    Wb = np.ascontiguousarray(
        np.concatenate([Wt, Wt], axis=0).astype(ml_dtypes.bfloat16)
    )
    g2 = np.ascontiguousarray(np.concatenate([gamma, gamma]))
    b2 = np.ascontiguousarray(np.concatenate([beta, beta]))
    e = np.eye(64, dtype=np.float32)
    eye4 = np.ascontiguousarray(np.block([[e, e], [e, e]]))
    return [
        {
            "I": I2[c * NS : (c + 1) * NS],
            "Wb": Wb,
            "gamma2": g2,
            "beta2": b2,
            "eye4": eye4,
        }
        for c in range(NCORES)
    ]


def kernel(I, gamma, beta, W, alpha):
    from concourse.bass_utils import run_bass_kernel_spmd

    nc = _get_nc()
    in_maps = _make_in_maps(I, gamma, beta, W, alpha)
    res = run_bass_kernel_spmd(nc, in_maps, list(range(NCORES)))
    out = np.concatenate(
        [np.asarray(res.results[c]["out"]) for c in range(NCORES)], axis=0
    )
    return out.astype(np.float32)


# revision 32
# speedup vs baseline: 1.0286x; 1.0286x over previous
"""BN(train) -> binarize -> conv1d(K=7,pad=3) -> alpha-scale -> maxpool2 on 8 trn2 cores.

Data-parallel over batch N: each core owns 8 samples. BN statistics are
computed per-core with bn_stats/bn_aggr; per-core (mean, E[x^2]) partials
([128,2] fp32) are exchanged with a tiny AllGather, then folded locally:
an exact fp32 matmul against a stacked-identity matrix sums the two
partition halves and broadcasts the result to all 128 partitions in one
shot. alpha is folded into the conv weights on the host.

Per-core layout: each sample is held in SBUF as [128, 4102] fp32 where
partitions 0-63 = channels for L 0..4095 (+halo) and partitions 64-127 =
channels for L 4096..8191 (+halo). The binarize is one ScalarE Sign op
whose output access pattern de-interleaves even/odd positions
(xb = [evens | odds]), so the conv can accumulate even-position and
odd-position outputs into separate PSUM banks from contiguous rhs slices.
The maxpool then collapses to a single VectorE tensor_tensor max over a
bank pair, written directly as bf16; outputs are stored as bf16 and
widened to fp32 on the host.
"""

import numpy as np
from contextlib import ExitStack

N, CIN, L = 64, 64, 8192
COUT, K = 128, 7
NCORES = 8
NS = N // NCORES          # samples per core
HALF = L // 2             # 4096
PAD = K // 2              # 3
WCOLS = HALF + 2 * PAD    # 4102
PHALF = WCOLS // 2        # 2051 (parity-split width incl. halo)
T = 512                   # conv output cols per matmul (= PSUM bank)
NT = HALF // (2 * T)      # 4 tile groups per half (each covers 1024 cols)
POOL_HALF = HALF // 2     # 2048 pooled cols per half
EPS = 1e-5

_CACHE = {}


def _build():
    import concourse.bass as bass
    import concourse.tile as tile
    from concourse import mybir

    f32 = mybir.dt.float32
    bf16 = mybir.dt.bfloat16
    Alu = mybir.AluOpType
    Act = mybir.ActivationFunctionType

    nc = bass.Bass()
    # I pre-transposed on host to [NS, 128, HALF]: row h*64+c holds
    # I[n, c, h*HALF:(h+1)*HALF]. 2D regular 128-row DRAM loads hit
    # fabric rate on the HWDGE path.
    I_h = nc.declare_dram_parameter("I", [NS, 128, HALF], f32, isOutput=False)
    # host-prepped: [128, K, COUT] bf16, channels duplicated on both halves,
    # alpha folded in
    Wb_h = nc.declare_dram_parameter("Wb", [128, K, COUT], bf16, isOutput=False)
    # gamma/beta duplicated to [128] on host
    g_h = nc.declare_dram_parameter("gamma2", [128], f32, isOutput=False)
    be_h = nc.declare_dram_parameter("beta2", [128], f32, isOutput=False)
    # [128,128] fp32: eye4[c,m] = 1 iff c % 64 == m % 64 (sums the two
    # partition halves and broadcasts to both halves, exactly, in fp32)
    eye_h = nc.declare_dram_parameter("eye4", [128, 128], f32, isOutput=False)
    out_h = nc.declare_dram_parameter("out", [NS, COUT, HALF], bf16, isOutput=True)

    with ExitStack() as ctx:
        tc = ctx.enter_context(tile.TileContext(nc))
        singles = ctx.enter_context(tc.tile_pool(name="singles", bufs=1))
        ibufs = ctx.enter_context(tc.tile_pool(name="ibufs", bufs=NS))
        xbs = ctx.enter_context(tc.tile_pool(name="xbs", bufs=3))
        psums = ctx.enter_context(tc.tile_pool(name="psums", bufs=2, space="PSUM"))
        stages = ctx.enter_context(tc.tile_pool(name="stages", bufs=4))
        dram = ctx.enter_context(tc.tile_pool(name="dram", bufs=1, space="DRAM"))

        # --- warmup AllGather, first thing on the CC stream: pays the
        # ncfw cold-boot + cross-core execution-start skew in the shadow of
        # the input loads. The input DRAM tile is deliberately
        # uninitialized — the gathered values are never read — so the
        # doorbell has no dependencies and rings immediately.
        agw_in = dram.tile([128, 2], f32)
        agw_out = dram.tile([NCORES * 128, 2], f32, addr_space="Shared")
        nc.gpsimd.collective_compute(
            "AllGather",
            Alu.bypass,
            replica_groups=[list(range(NCORES))],
            ins=[agw_in[:]],
            outs=[agw_out[:]],
        )

        eps_t = singles.tile([128, 1], f32)
        nc.vector.memset(eps_t[:], EPS)

        # --- phase 1: load + per-partition stats -------------------------
        # 512 KB chunks so the first stats tiles land early and the tail
        # past the last chunk is short. The 64 stat tiles are split
        # between VectorE (bn_stats, 44 tiles) and the otherwise-idle
        # ScalarE (20 tiles via two activation passes whose accum_out
        # gives per-partition sums: Square -> sum(x^2), Copy -> sum(x)).
        # ScalarE's stats share is small and front-loaded on early samples:
        # its first ~6 us are spent issuing its DMA ring, so late-sample
        # tiles belong to VectorE which drains them as they land.
        NSC_PER = [3, 3, 2, 2, 1, 1, 0, 0]     # scalar-owned tiles/sample
        NSC = sum(NSC_PER)                      # 12
        NDVE = NS * 8 - NSC                     # 52
        stats = singles.tile([128, NDVE, 6], f32)
        sq_acc = singles.tile([128, 2, NSC], f32)   # [;(sumsq,sum); tile]
        scst = ctx.enter_context(tc.tile_pool(name="scst", bufs=2))
        # all load issues first, so neither HWDGE ring is ever starved
        # behind compute instructions on its issuing engine
        ibs = []
        for n in range(NS):
            ib = ibufs.tile([128, WCOLS], f32, name=f"ib{n}", tag="ib")
            ibs.append(ib)
            # small chunks for the first two samples (early stats start),
            # 1 MB chunks for the bulk (better DMA efficiency)
            bounds = [0, 1024, 2048, 3072, 4096] if n < 2 else [0, 2048, 4096]
            for h, (b0, b1) in enumerate(zip(bounds, bounds[1:])):
                eng = nc.sync if (n + h) % 2 == 0 else nc.scalar
                eng.dma_start(
                    out=ib[:, PAD + b0 : PAD + b1],
                    in_=I_h[n, :, b0:b1],
                )
        # constants on the gpsimd/SWDGE queue (not needed until the stats
        # fold / conv phase)
        wsb = singles.tile([128, K, COUT], bf16)
        nc.gpsimd.dma_start(out=wsb[:], in_=Wb_h[:])
        eye4 = singles.tile([128, 128], f32)
        nc.gpsimd.dma_start(out=eye4[:], in_=eye_h[:])
        gam = singles.tile([128, 1], f32)
        bet = singles.tile([128, 1], f32)
        nc.gpsimd.dma_start(out=gam[:], in_=g_h[:].rearrange("(c o) -> c o", o=1))
        nc.gpsimd.dma_start(out=bet[:], in_=be_h[:].rearrange("(c o) -> c o", o=1))
        for n in range(NS):
            ib = ibs[n]
            # halo: lo rows need positions 4096..4098 (hi rows, first 3),
            # hi rows need 4093..4095 (lo rows, last 3)
            nc.gpsimd.dma_start(
                out=ib[0:CIN, WCOLS - PAD : WCOLS], in_=I_h[n, CIN:128, 0:PAD]
            )
            nc.gpsimd.dma_start(
                out=ib[CIN:128, 0:PAD], in_=I_h[n, 0:CIN, HALF - PAD : HALF]
            )
        idve = isc = 0
        for n in range(NS):
            ib = ibs[n]
            nsc = NSC_PER[n]
            for c in range(8):
                tl = ib[:, PAD + c * T : PAD + (c + 1) * T]
                if c < nsc:
                    scr = scst.tile([128, T], bf16, name="scr")
                    nc.scalar.activation(
                        out=scr[:], in_=tl, func=Act.Copy,
                        accum_out=sq_acc[:, 0, isc : isc + 1],
                    )
                    nc.scalar.activation(
                        out=scr[:], in_=tl, func=Act.Square,
                        accum_out=sq_acc[:, 1, isc : isc + 1],
                    )
                    isc += 1
                else:
                    nc.vector.bn_stats(out=stats[:, idve, :], in_=tl)
                    idve += 1

        # --- fold per-core stats -> raw (sum, sumsq) [128,2], AllGather --
        mv = singles.tile([128, 2], f32)
        nc.vector.bn_aggr(out=mv[:], in_=stats[:])
        tmp0 = singles.tile([128, 1], f32)
        nc.vector.tensor_tensor(
            out=tmp0[:], in0=mv[:, 0:1], in1=mv[:, 0:1], op=Alu.mult
        )
        # mv becomes (mean, E[x^2]) over the DVE-owned tiles
        nc.vector.tensor_tensor(
            out=mv[:, 1:2], in0=mv[:, 1:2], in1=tmp0[:], op=Alu.add
        )
        qs_sc = singles.tile([128, 2], f32)
        nc.vector.tensor_reduce(
            out=qs_sc[:], in_=sq_acc[:], axis=mybir.AxisListType.X, op=Alu.add
        )
        # ms = raw (sum, sumsq) = (mean, Ex2)*Ndve*T + (sum_sc, sumsq_sc)
        ms = singles.tile([128, 2], f32)
        nc.vector.scalar_tensor_tensor(
            out=ms[:],
            in0=mv[:],
            scalar=float(NDVE * T),
            in1=qs_sc[:],
            op0=Alu.mult,
            op1=Alu.add,
        )
        ag_in = dram.tile([128, 2], f32)
        ag_out = dram.tile([NCORES * 128, 2], f32, addr_space="Shared")
        nc.sync.dma_start(out=ag_in[:], in_=ms[:])
        nc.gpsimd.collective_compute(
            "AllGather",
            Alu.bypass,
            replica_groups=[list(range(NCORES))],
            ins=[ag_in[:]],
            outs=[ag_out[:]],
        )
        ag_sb = singles.tile([128, 2, NCORES], f32)
        nc.sync.dma_start(
            out=ag_sb[:], in_=ag_out[:].rearrange("(r p) t -> p t r", r=NCORES)
        )
        s8 = singles.tile([128, 2], f32)
        nc.vector.tensor_reduce(
            out=s8[:], in_=ag_sb[:], axis=mybir.AxisListType.X, op=Alu.add
        )
        # exact fp32 fold of the two partition halves, broadcast to both.
        # Borrows a conv-pool PSUM slot; it is read (and released) before
        # the first conv matmul needs it.
        q_tile = psums.tile([COUT, T], f32, name="ps_loe")
        q_ps = q_tile[:, 0:2]
        nc.tensor.matmul(q_ps, eye4[:], s8[:], start=True, stop=True)

        # --- global stats -> binarize scale/bias -------------------------
        q = singles.tile([128, 2], f32)
        # q = (mean, E[x^2]) from the raw global sums over N*L elements
        nc.vector.tensor_scalar_mul(q[:], q_ps, 1.0 / float(N * L))
        sb = singles.tile([128, 2], f32)
        tmp = singles.tile([128, 2], f32)
        var = singles.tile([128, 1], f32)
        rstd = singles.tile([128, 1], f32)
        nc.vector.tensor_tensor(
            out=tmp[:, 0:1], in0=q[:, 0:1], in1=q[:, 0:1], op=Alu.mult
        )
        nc.vector.tensor_tensor(
            out=var[:], in0=q[:, 1:2], in1=tmp[:, 0:1], op=Alu.subtract
        )
        nc.scalar.activation(
            out=rstd[:], in_=var[:], func=Act.Sqrt, bias=eps_t[:], scale=1.0
        )
        nc.vector.reciprocal(out=rstd[:], in_=rstd[:])
        # s = gamma*rstd ; b = beta - mean*s
        nc.vector.tensor_tensor(out=sb[:, 0:1], in0=gam[:], in1=rstd[:], op=Alu.mult)
        nc.vector.tensor_tensor(
            out=tmp[:, 1:2], in0=q[:, 0:1], in1=sb[:, 0:1], op=Alu.mult
        )
        nc.vector.tensor_tensor(
            out=sb[:, 1:2], in0=bet[:], in1=tmp[:, 1:2], op=Alu.subtract
        )

        # --- phase 2: binarize (parity-split) + conv + pool --------------
        for n in range(NS):
            ib = ibs[n]
            # xb layout per partition: [evens (2051) | odds (2051)], bf16.
            # Activations use stride-2 single-dim input views and
            # contiguous outputs (a 2-element inner AP dim costs ~9
            # cycles/row on the engines; a flat strided dim is full rate).
            # Sample 0 is on the critical path out of the stats fold, so
            # its binarize is split into E/O column chunks — the first
            # conv tiles only need the leading columns of each parity.
            xb = xbs.tile([128, 2 * PHALF], bf16, name="xb")
            ib_par = ib[:].rearrange("p (w two) -> p two w", two=2)
            csplits = [0, 516, 1028, 1540, PHALF] if n == 0 else [0, PHALF]
            for c0, c1 in zip(csplits, csplits[1:]):
                for par in range(2):
                    nc.scalar.activation(
                        out=xb[:, par * PHALF + c0 : par * PHALF + c1],
                        in_=ib_par[:, par, c0:c1],
                        func=Act.Sign,
                        bias=sb[:, 1:2],
                        scale=sb[:, 0:1],
                    )
            # zero the off-the-end pads (binarize mapped them to +-1):
            # lo rows: padded p in {0,1,2} -> E0, E1 (cols 0:2) and O0 (2051)
            # hi rows: padded p in {4099,4100,4101} -> E2050 (2050) and
            #          O2049, O2050 (cols 4100:4102)
            nc.gpsimd.memset(xb[0:CIN, 0:2], 0.0)
            nc.gpsimd.memset(xb[0:CIN, PHALF : PHALF + 1], 0.0)
            nc.gpsimd.memset(xb[CIN:128, PHALF - 1 : PHALF], 0.0)
            nc.gpsimd.memset(xb[CIN:128, 2 * PHALF - 2 : 2 * PHALF], 0.0)

            for j in range(NT):
                m0 = j * T
                ps_loe = psums.tile([COUT, T], f32, name="ps_loe")
                ps_loo = psums.tile([COUT, T], f32, name="ps_loo")
                ps_hie = psums.tile([COUT, T], f32, name="ps_hie")
                ps_hio = psums.tile([COUT, T], f32, name="ps_hio")
                for k in range(K):
                    st = k == 0
                    sp = k == K - 1
                    # even outputs c=2m: tap k reads parity (k%2) at
                    # offset k//2; odd outputs c=2m+1: parity (k+1)%2 at
                    # offset (k+1)//2. Same-weight matmuls are adjacent
                    # (e then o per row group) so the weight load can be
                    # shared/dedued by the backend.
                    eo = (k % 2) * PHALF + k // 2 + m0
                    oo = ((k + 1) % 2) * PHALF + (k + 1) // 2 + m0
                    nc.tensor.matmul(
                        ps_loe[:], wsb[0:CIN, k, :], xb[0:CIN, eo : eo + T],
                        start=st, stop=sp,
                    )
                    nc.tensor.matmul(
                        ps_loo[:], wsb[0:CIN, k, :], xb[0:CIN, oo : oo + T],
                        start=st, stop=sp,
                    )
                    nc.tensor.matmul(
                        ps_hie[:], wsb[CIN:128, k, :], xb[CIN:128, eo : eo + T],
                        start=st, stop=sp,
                    )
                    nc.tensor.matmul(
                        ps_hio[:], wsb[CIN:128, k, :], xb[CIN:128, oo : oo + T],
                        start=st, stop=sp,
                    )
                # walrus only allows one PSUM input per DVE op: ScalarE
                # evacuates the even banks (ACT is fast at PSUM reads), the
                # max then reads one SBUF + one PSUM operand on VectorE.
                ev_lo = stages.tile([COUT, T], f32, name="ev_lo")
                ev_hi = stages.tile([COUT, T], f32, name="ev_hi")
                if n == 0:
                    # ScalarE is still busy binarizing early on; keep the
                    # first sample's bank recycling off its queue
                    nc.vector.tensor_copy(out=ev_lo[:], in_=ps_loe[:])
                    nc.vector.tensor_copy(out=ev_hi[:], in_=ps_hie[:])
                else:
                    nc.scalar.activation(
                        out=ev_lo[:], in_=ps_loe[:], func=Act.Copy
                    )
                    nc.scalar.activation(
                        out=ev_hi[:], in_=ps_hie[:], func=Act.Copy
                    )
                st_lo = stages.tile([COUT, T], bf16, name="st_lo")
                st_hi = stages.tile([COUT, T], bf16, name="st_hi")
                nc.vector.tensor_tensor(
                    out=st_lo[:], in0=ev_lo[:], in1=ps_loo[:], op=Alu.max
                )
                nc.vector.tensor_tensor(
                    out=st_hi[:], in0=ev_hi[:], in1=ps_hio[:], op=Alu.max
                )
                nc.sync.dma_start(
                    out=out_h[n, :, j * T : (j + 1) * T], in_=st_lo[:]
                )
                nc.sync.dma_start(
                    out=out_h[n, :, POOL_HALF + j * T : POOL_HALF + (j + 1) * T],
                    in_=st_hi[:],
                )

    return nc


def _split_multi_waits(nc):
    """walrus codegen only supports one sync-wait command per instruction;
    the TileContext exit drain carries several. Split the extras onto NOPs
    inserted immediately before the offending instruction."""
    import bass_rust
    from concourse import mybir

    for f in nc.m.functions:
        for bb in f.blocks:
            idx = 0
            while idx < len(bb.instructions):
                ins = bb.instructions[idx]
                si = ins.sync_info
                if si is not None and si.on_wait and len(si.on_wait) > 1:
                    waits = list(si.on_wait)
                    keep, rest = waits[-1], waits[:-1]
                    ins.sync_info = bass_rust.SyncInfo(
                        on_wait=[keep], on_update=list(si.on_update or [])
                    )
                    new_insts = []
                    for w in rest:
                        nop = mybir.InstNoOp(
                            name=nc.get_next_instruction_name(), ins=[], outs=[]
                        )
                        nop.engine = ins.engine
                        nop.sync_info = bass_rust.SyncInfo(on_wait=[w], on_update=[])
                        new_insts.append(nop)
                    for j, nop in enumerate(new_insts):
                        bb.instructions.insert(idx + j, nop)
                    idx += len(new_insts)
                idx += 1


def _get_nc(split=True):
    key = ("nc", split)
    if key not in _CACHE:
        nc = _build()
        if split:
            _split_multi_waits(nc)
        _CACHE[key] = nc
    return _CACHE[key]


def _make_in_maps(I, gamma, beta, W, alpha):
    import ml_dtypes

    I = np.asarray(I, dtype=np.float32)
    gamma = np.ascontiguousarray(np.asarray(gamma, dtype=np.float32))
    beta = np.ascontiguousarray(np.asarray(beta, dtype=np.float32))
    W = np.asarray(W, dtype=np.float32)
    alpha = np.asarray(alpha, dtype=np.float32)

    # stack the two L-halves on the partition axis: [N, 128, HALF]
    I2 = np.ascontiguousarray(
        I.reshape(N, CIN, 2, HALF).transpose(0, 2, 1, 3).reshape(N, 128, HALF)
    )
    # fold alpha into the weights, arrange [CIN, K, COUT], duplicate the
    # channel block on both partition halves, cast bf16
    Wt = (W * alpha.reshape(COUT, 1, 1)).transpose(1, 2, 0)  # [CIN, K, COUT]
    Wb = np.ascontiguousarray(
        np.concatenate([Wt, Wt], axis=0).astype(ml_dtypes.bfloat16)
    )
    g2 = np.ascontiguousarray(np.concatenate([gamma, gamma]))
    b2 = np.ascontiguousarray(np.concatenate([beta, beta]))
    e = np.eye(64, dtype=np.float32)
    eye4 = np.ascontiguousarray(np.block([[e, e], [e, e]]))
    return [
        {
            "I": I2[c * NS : (c + 1) * NS],
            "Wb": Wb,
            "gamma2": g2,
            "beta2": b2,
            "eye4": eye4,
        }
        for c in range(NCORES)
    ]


def kernel(I, gamma, beta, W, alpha):
    from concourse.bass_utils import run_bass_kernel_spmd

    nc = _get_nc()
    in_maps = _make_in_maps(I, gamma, beta, W, alpha)
    res = run_bass_kernel_spmd(nc, in_maps, list(range(NCORES)))
    out = np.concatenate(
        [np.asarray(res.results[c]["out"]) for c in range(NCORES)], axis=0
    )
    return out.astype(np.float32)


# revision 33
# speedup vs baseline: 1.0309x; 1.0022x over previous
"""BN(train) -> binarize -> conv1d(K=7,pad=3) -> alpha-scale -> maxpool2 on 8 trn2 cores.

Data-parallel over batch N: each core owns 8 samples. BN statistics are
computed per-core with bn_stats/bn_aggr; per-core (mean, E[x^2]) partials
([128,2] fp32) are exchanged with a tiny AllGather, then folded locally:
an exact fp32 matmul against a stacked-identity matrix sums the two
partition halves and broadcasts the result to all 128 partitions in one
shot. alpha is folded into the conv weights on the host.

Per-core layout: each sample is held in SBUF as [128, 4102] fp32 where
partitions 0-63 = channels for L 0..4095 (+halo) and partitions 64-127 =
channels for L 4096..8191 (+halo). The binarize is one ScalarE Sign op
whose output access pattern de-interleaves even/odd positions
(xb = [evens | odds]), so the conv can accumulate even-position and
odd-position outputs into separate PSUM banks from contiguous rhs slices.
The maxpool then collapses to a single VectorE tensor_tensor max over a
bank pair, written directly as bf16; outputs are stored as bf16 and
widened to fp32 on the host.
"""

import numpy as np
from contextlib import ExitStack

N, CIN, L = 64, 64, 8192
COUT, K = 128, 7
NCORES = 8
NS = N // NCORES          # samples per core
HALF = L // 2             # 4096
PAD = K // 2              # 3
WCOLS = HALF + 2 * PAD    # 4102
PHALF = WCOLS // 2        # 2051 (parity-split width incl. halo)
T = 512                   # conv output cols per matmul (= PSUM bank)
NT = HALF // (2 * T)      # 4 tile groups per half (each covers 1024 cols)
POOL_HALF = HALF // 2     # 2048 pooled cols per half
EPS = 1e-5

_CACHE = {}


def _build():
    import concourse.bass as bass
    import concourse.tile as tile
    from concourse import mybir

    f32 = mybir.dt.float32
    bf16 = mybir.dt.bfloat16
    Alu = mybir.AluOpType
    Act = mybir.ActivationFunctionType

    nc = bass.Bass()
    # I pre-transposed on host to [NS, 128, HALF]: row h*64+c holds
    # I[n, c, h*HALF:(h+1)*HALF]. 2D regular 128-row DRAM loads hit
    # fabric rate on the HWDGE path.
    I_h = nc.declare_dram_parameter("I", [NS, 128, HALF], f32, isOutput=False)
    # host-prepped: [128, K, COUT] bf16, channels duplicated on both halves,
    # alpha folded in
    Wb_h = nc.declare_dram_parameter("Wb", [128, K, COUT], bf16, isOutput=False)
    # gamma/beta duplicated to [128] on host
    g_h = nc.declare_dram_parameter("gamma2", [128], f32, isOutput=False)
    be_h = nc.declare_dram_parameter("beta2", [128], f32, isOutput=False)
    # [128,128] fp32: eye4[c,m] = 1 iff c % 64 == m % 64 (sums the two
    # partition halves and broadcasts to both halves, exactly, in fp32)
    eye_h = nc.declare_dram_parameter("eye4", [128, 128], f32, isOutput=False)
    out_h = nc.declare_dram_parameter("out", [NS, COUT, HALF], bf16, isOutput=True)

    with ExitStack() as ctx:
        tc = ctx.enter_context(tile.TileContext(nc))
        singles = ctx.enter_context(tc.tile_pool(name="singles", bufs=1))
        ibufs = ctx.enter_context(tc.tile_pool(name="ibufs", bufs=NS))
        xbs = ctx.enter_context(tc.tile_pool(name="xbs", bufs=3))
        psums = ctx.enter_context(tc.tile_pool(name="psums", bufs=2, space="PSUM"))
        stages = ctx.enter_context(tc.tile_pool(name="stages", bufs=4))
        dram = ctx.enter_context(tc.tile_pool(name="dram", bufs=1, space="DRAM"))

        # --- warmup AllGather, first thing on the CC stream: pays the
        # ncfw cold-boot + cross-core execution-start skew in the shadow of
        # the input loads. The input DRAM tile is deliberately
        # uninitialized — the gathered values are never read — so the
        # doorbell has no dependencies and rings immediately.
        agw_in = dram.tile([128, 2], f32)
        agw_out = dram.tile([NCORES * 128, 2], f32, addr_space="Shared")
        nc.gpsimd.collective_compute(
            "AllGather",
            Alu.bypass,
            replica_groups=[list(range(NCORES))],
            ins=[agw_in[:]],
            outs=[agw_out[:]],
        )

        eps_t = singles.tile([128, 1], f32)
        nc.vector.memset(eps_t[:], EPS)

        # --- phase 1: load + per-partition stats -------------------------
        # 512 KB chunks so the first stats tiles land early and the tail
        # past the last chunk is short. The 64 stat tiles are split
        # between VectorE (bn_stats, 44 tiles) and the otherwise-idle
        # ScalarE (20 tiles via two activation passes whose accum_out
        # gives per-partition sums: Square -> sum(x^2), Copy -> sum(x)).
        # ScalarE's stats share is small and front-loaded on early samples:
        # its first ~6 us are spent issuing its DMA ring, so late-sample
        # tiles belong to VectorE which drains them as they land.
        NSC_PER = [3, 3, 2, 2, 1, 1, 0, 0]     # scalar-owned tiles/sample
        NSC = sum(NSC_PER)                      # 12
        NDVE = NS * 8 - NSC                     # 52
        stats = singles.tile([128, NDVE, 6], f32)
        sq_acc = singles.tile([128, 2, NSC], f32)   # [;(sumsq,sum); tile]
        scst = ctx.enter_context(tc.tile_pool(name="scst", bufs=2))
        # all load issues first, so neither HWDGE ring is ever starved
        # behind compute instructions on its issuing engine
        ibs = []
        for n in range(NS):
            ib = ibufs.tile([128, WCOLS], f32, name=f"ib{n}", tag="ib")
            ibs.append(ib)
            # small chunks for the first two samples (early stats start),
            # 1 MB chunks for the bulk (better DMA efficiency)
            bounds = [0, 1024, 2048, 3072, 4096] if n < 2 else [0, 2048, 4096]
            for h, (b0, b1) in enumerate(zip(bounds, bounds[1:])):
                eng = nc.sync if (n + h) % 2 == 0 else nc.scalar
                eng.dma_start(
                    out=ib[:, PAD + b0 : PAD + b1],
                    in_=I_h[n, :, b0:b1],
                )
        # constants on the gpsimd/SWDGE queue (not needed until the stats
        # fold / conv phase)
        wsb = singles.tile([128, K, COUT], bf16)
        nc.gpsimd.dma_start(out=wsb[:], in_=Wb_h[:])
        eye4 = singles.tile([128, 128], f32)
        nc.gpsimd.dma_start(out=eye4[:], in_=eye_h[:])
        gam = singles.tile([128, 1], f32)
        bet = singles.tile([128, 1], f32)
        nc.gpsimd.dma_start(out=gam[:], in_=g_h[:].rearrange("(c o) -> c o", o=1))
        nc.gpsimd.dma_start(out=bet[:], in_=be_h[:].rearrange("(c o) -> c o", o=1))
        for n in range(NS):
            ib = ibs[n]
            # halo: lo rows need positions 4096..4098 (hi rows, first 3),
            # hi rows need 4093..4095 (lo rows, last 3)
            nc.gpsimd.dma_start(
                out=ib[0:CIN, WCOLS - PAD : WCOLS], in_=I_h[n, CIN:128, 0:PAD]
            )
            nc.gpsimd.dma_start(
                out=ib[CIN:128, 0:PAD], in_=I_h[n, 0:CIN, HALF - PAD : HALF]
            )
        idve = isc = 0
        for n in range(NS):
            ib = ibs[n]
            nsc = NSC_PER[n]
            for c in range(8):
                tl = ib[:, PAD + c * T : PAD + (c + 1) * T]
                if c < nsc:
                    scr = scst.tile([128, T], bf16, name="scr")
                    nc.scalar.activation(
                        out=scr[:], in_=tl, func=Act.Copy,
                        accum_out=sq_acc[:, 0, isc : isc + 1],
                    )
                    nc.scalar.activation(
                        out=scr[:], in_=tl, func=Act.Square,
                        accum_out=sq_acc[:, 1, isc : isc + 1],
                    )
                    isc += 1
                else:
                    nc.vector.bn_stats(out=stats[:, idve, :], in_=tl)
                    idve += 1

        # --- fold per-core stats -> raw (sum, sumsq) [128,2], AllGather --
        mv = singles.tile([128, 2], f32)
        nc.vector.bn_aggr(out=mv[:], in_=stats[:])
        tmp0 = singles.tile([128, 1], f32)
        nc.vector.tensor_tensor(
            out=tmp0[:], in0=mv[:, 0:1], in1=mv[:, 0:1], op=Alu.mult
        )
        # mv becomes (mean, E[x^2]) over the DVE-owned tiles
        nc.vector.tensor_tensor(
            out=mv[:, 1:2], in0=mv[:, 1:2], in1=tmp0[:], op=Alu.add
        )
        qs_sc = singles.tile([128, 2], f32)
        nc.vector.tensor_reduce(
            out=qs_sc[:], in_=sq_acc[:], axis=mybir.AxisListType.X, op=Alu.add
        )
        # ms = raw (sum, sumsq) = (mean, Ex2)*Ndve*T + (sum_sc, sumsq_sc)
        ms = singles.tile([128, 2], f32)
        nc.vector.scalar_tensor_tensor(
            out=ms[:],
            in0=mv[:],
            scalar=float(NDVE * T),
            in1=qs_sc[:],
            op0=Alu.mult,
            op1=Alu.add,
        )
        ag_in = dram.tile([128, 2], f32)
        ag_out = dram.tile([NCORES * 128, 2], f32, addr_space="Shared")
        nc.sync.dma_start(out=ag_in[:], in_=ms[:])
        nc.gpsimd.collective_compute(
            "AllGather",
            Alu.bypass,
            replica_groups=[list(range(NCORES))],
            ins=[ag_in[:]],
            outs=[ag_out[:]],
        )
        ag_sb = singles.tile([128, 2, NCORES], f32)
        nc.sync.dma_start(
            out=ag_sb[:], in_=ag_out[:].rearrange("(r p) t -> p t r", r=NCORES)
        )
        s8 = singles.tile([128, 2], f32)
        nc.vector.tensor_reduce(
            out=s8[:], in_=ag_sb[:], axis=mybir.AxisListType.X, op=Alu.add
        )
        # exact fp32 fold of the two partition halves, broadcast to both.
        # Borrows a conv-pool PSUM slot; it is read (and released) before
        # the first conv matmul needs it.
        q_tile = psums.tile([COUT, T], f32, name="ps_loe")
        q_ps = q_tile[:, 0:2]
        nc.tensor.matmul(q_ps, eye4[:], s8[:], start=True, stop=True)

        # --- global stats -> binarize scale/bias -------------------------
        q = singles.tile([128, 2], f32)
        # q = (mean, E[x^2]) from the raw global sums over N*L elements
        nc.vector.tensor_scalar_mul(q[:], q_ps, 1.0 / float(N * L))
        sb = singles.tile([128, 2], f32)
        tmp = singles.tile([128, 2], f32)
        var = singles.tile([128, 1], f32)
        rstd = singles.tile([128, 1], f32)
        nc.vector.tensor_tensor(
            out=tmp[:, 0:1], in0=q[:, 0:1], in1=q[:, 0:1], op=Alu.mult
        )
        nc.vector.tensor_tensor(
            out=var[:], in0=q[:, 1:2], in1=tmp[:, 0:1], op=Alu.subtract
        )
        nc.scalar.activation(
            out=rstd[:], in_=var[:], func=Act.Sqrt, bias=eps_t[:], scale=1.0
        )
        nc.vector.reciprocal(out=rstd[:], in_=rstd[:])
        # s = gamma*rstd ; b = beta - mean*s
        nc.vector.tensor_tensor(out=sb[:, 0:1], in0=gam[:], in1=rstd[:], op=Alu.mult)
        nc.vector.tensor_tensor(
            out=tmp[:, 1:2], in0=q[:, 0:1], in1=sb[:, 0:1], op=Alu.mult
        )
        nc.vector.tensor_tensor(
            out=sb[:, 1:2], in0=bet[:], in1=tmp[:, 1:2], op=Alu.subtract
        )

        # --- phase 2: binarize (parity-split) + conv + pool --------------
        for n in range(NS):
            ib = ibs[n]
            # xb layout per partition: [evens (2051) | odds (2051)], bf16.
            # Activations use stride-2 single-dim input views and
            # contiguous outputs (a 2-element inner AP dim costs ~9
            # cycles/row on the engines; a flat strided dim is full rate).
            # Sample 0 is on the critical path out of the stats fold, so
            # its binarize is split into E/O column chunks — the first
            # conv tiles only need the leading columns of each parity.
            xb = xbs.tile([128, 2 * PHALF], bf16, name="xb")
            ib_par = ib[:].rearrange("p (w two) -> p two w", two=2)
            csplits = [0, 516, 1028, 1540, PHALF] if n == 0 else [0, PHALF]
            for c0, c1 in zip(csplits, csplits[1:]):
                for par in range(2):
                    nc.scalar.activation(
                        out=xb[:, par * PHALF + c0 : par * PHALF + c1],
                        in_=ib_par[:, par, c0:c1],
                        func=Act.Sign,
                        bias=sb[:, 1:2],
                        scale=sb[:, 0:1],
                    )
            # zero the off-the-end pads (binarize mapped them to +-1):
            # lo rows: padded p in {0,1,2} -> E0, E1 (cols 0:2) and O0 (2051)
            # hi rows: padded p in {4099,4100,4101} -> E2050 (2050) and
            #          O2049, O2050 (cols 4100:4102)
            nc.gpsimd.memset(xb[0:CIN, 0:2], 0.0)
            nc.gpsimd.memset(xb[0:CIN, PHALF : PHALF + 1], 0.0)
            nc.gpsimd.memset(xb[CIN:128, PHALF - 1 : PHALF], 0.0)
            nc.gpsimd.memset(xb[CIN:128, 2 * PHALF - 2 : 2 * PHALF], 0.0)

            for j in range(NT):
                m0 = j * T
                ps_loe = psums.tile([COUT, T], f32, name="ps_loe")
                ps_loo = psums.tile([COUT, T], f32, name="ps_loo")
                ps_hie = psums.tile([COUT, T], f32, name="ps_hie")
                ps_hio = psums.tile([COUT, T], f32, name="ps_hio")
                for k in range(K):
                    st = k == 0
                    sp = k == K - 1
                    # even outputs c=2m: tap k reads parity (k%2) at
                    # offset k//2; odd outputs c=2m+1: parity (k+1)%2 at
                    # offset (k+1)//2. Same-weight matmuls are adjacent
                    # (e then o per row group) so the weight load can be
                    # shared/dedued by the backend.
                    eo = (k % 2) * PHALF + k // 2 + m0
                    oo = ((k + 1) % 2) * PHALF + (k + 1) // 2 + m0
                    nc.tensor.matmul(
                        ps_loe[:], wsb[0:CIN, k, :], xb[0:CIN, eo : eo + T],
                        start=st, stop=sp,
                    )
                    nc.tensor.matmul(
                        ps_loo[:], wsb[0:CIN, k, :], xb[0:CIN, oo : oo + T],
                        start=st, stop=sp,
                    )
                    nc.tensor.matmul(
                        ps_hie[:], wsb[CIN:128, k, :], xb[CIN:128, eo : eo + T],
                        start=st, stop=sp,
                    )
                    nc.tensor.matmul(
                        ps_hio[:], wsb[CIN:128, k, :], xb[CIN:128, oo : oo + T],
                        start=st, stop=sp,
                    )
                # walrus only allows one PSUM input per DVE op: ScalarE
                # evacuates the even banks (ACT is fast at PSUM reads), the
                # max then reads one SBUF + one PSUM operand on VectorE.
                ev_lo = stages.tile([COUT, T], f32, name="ev_lo")
                ev_hi = stages.tile([COUT, T], f32, name="ev_hi")
                if n == 0:
                    # ScalarE is still busy binarizing early on; keep the
                    # first sample's bank recycling off its queue
                    nc.vector.tensor_copy(out=ev_lo[:], in_=ps_loe[:])
                    nc.vector.tensor_copy(out=ev_hi[:], in_=ps_hie[:])
                elif n == NS - 1:
                    # last sample: evacuate on both engines in parallel so
                    # the post-final-matmul tail (evac -> max -> store) is
                    # as short as possible
                    nc.scalar.activation(
                        out=ev_lo[:], in_=ps_loe[:], func=Act.Copy
                    )
                    nc.vector.tensor_copy(out=ev_hi[:], in_=ps_hie[:])
                else:
                    nc.scalar.activation(
                        out=ev_lo[:], in_=ps_loe[:], func=Act.Copy
                    )
                    nc.scalar.activation(
                        out=ev_hi[:], in_=ps_hie[:], func=Act.Copy
                    )
                st_lo = stages.tile([COUT, T], bf16, name="st_lo")
                st_hi = stages.tile([COUT, T], bf16, name="st_hi")
                nc.vector.tensor_tensor(
                    out=st_lo[:], in0=ev_lo[:], in1=ps_loo[:], op=Alu.max
                )
                nc.vector.tensor_tensor(
                    out=st_hi[:], in0=ev_hi[:], in1=ps_hio[:], op=Alu.max
                )
                nc.sync.dma_start(
                    out=out_h[n, :, j * T : (j + 1) * T], in_=st_lo[:]
                )
                nc.sync.dma_start(
                    out=out_h[n, :, POOL_HALF + j * T : POOL_HALF + (j + 1) * T],
                    in_=st_hi[:],
                )

    return nc


def _split_multi_waits(nc):
    """walrus codegen only supports one sync-wait command per instruction;
    the TileContext exit drain carries several. Split the extras onto NOPs
    inserted immediately before the offending instruction."""
    import bass_rust
    from concourse import mybir

    for f in nc.m.functions:
        for bb in f.blocks:
            idx = 0
            while idx < len(bb.instructions):
                ins = bb.instructions[idx]
                si = ins.sync_info
                if si is not None and si.on_wait and len(si.on_wait) > 1:
                    waits = list(si.on_wait)
                    keep, rest = waits[-1], waits[:-1]
                    ins.sync_info = bass_rust.SyncInfo(
                        on_wait=[keep], on_update=list(si.on_update or [])
                    )
                    new_insts = []
                    for w in rest:
                        nop = mybir.InstNoOp(
                            name=nc.get_next_instruction_name(), ins=[], outs=[]
                        )
                        nop.engine = ins.engine
                        nop.sync_info = bass_rust.SyncInfo(on_wait=[w], on_update=[])
                        new_insts.append(nop)
                    for j, nop in enumerate(new_insts):
                        bb.instructions.insert(idx + j, nop)
                    idx += len(new_insts)
                idx += 1


def _get_nc(split=True):
    key = ("nc", split)
    if key not in _CACHE:
        nc = _build()
        if split:
            _split_multi_waits(nc)
        _CACHE[key] = nc
    return _CACHE[key]


def _make_in_maps(I, gamma, beta, W, alpha):
    import ml_dtypes

    I = np.asarray(I, dtype=np.float32)
    gamma = np.ascontiguousarray(np.asarray(gamma, dtype=np.float32))
    beta = np.ascontiguousarray(np.asarray(beta, dtype=np.float32))
    W = np.asarray(W, dtype=np.float32)
    alpha = np.asarray(alpha, dtype=np.float32)

    # stack the two L-halves on the partition axis: [N, 128, HALF]
    I2 = np.ascontiguousarray(
        I.reshape(N, CIN, 2, HALF).transpose(0, 2, 1, 3).reshape(N, 128, HALF)
    )
    # fold alpha into the weights, arrange [CIN, K, COUT], duplicate the
    # channel block on both partition halves, cast bf16
    Wt = (W * alpha.reshape(COUT, 1, 1)).transpose(1, 2, 0)  # [CIN, K, COUT]
    Wb = np.ascontiguousarray(
        np.concatenate([Wt, Wt], axis=0).astype(ml_dtypes.bfloat16)
    )
    g2 = np.ascontiguousarray(np.concatenate([gamma, gamma]))
    b2 = np.ascontiguousarray(np.concatenate([beta, beta]))
    e = np.eye(64, dtype=np.float32)
    eye4 = np.ascontiguousarray(np.block([[e, e], [e, e]]))
    return [
        {
            "I": I2[c * NS : (c + 1) * NS],
            "Wb": Wb,
            "gamma2": g2,
            "beta2": b2,
            "eye4": eye4,
        }
        for c in range(NCORES)
    ]


def kernel(I, gamma, beta, W, alpha):
    from concourse.bass_utils import run_bass_kernel_spmd

    nc = _get_nc()
    in_maps = _make_in_maps(I, gamma, beta, W, alpha)
    res = run_bass_kernel_spmd(nc, in_maps, list(range(NCORES)))
    out = np.concatenate(
        [np.asarray(res.results[c]["out"]) for c in range(NCORES)], axis=0
    )
    return out.astype(np.float32)
